# revision 23
# baseline (speedup 1.0000x reference)
"""Canny edge detection on TRN2 — fused-DVE redesign.

Geometry (per core), same as the baseline:
  - Output rows: rows_out (2048) of the tall (B*H, W) image.
  - Tile t reads xs rows [120t, 120t+128) (xs row 0 = tall row R0-6),
    valid NMS rows on partitions p in [2,122) -> strip rows 120t + p - 2.
  - Hysteresis on a vertically bit-packed strip: 24 rows per int32 word.

Pipeline (per tile):
  - DVE fused U8FLOOR: u = floor(fl(fl(x+1)*127.5))            (bf16)
  - PE: s = conv2d(u, Kx+Ky), d = conv2d(u, Kx-Ky) per channel, via 3
    column-shifted matmuls each into PSUM (vertical taps in the
    stationary, horizontal taps via shifted rhs views).
  - Act: copy s,d PSUM -> SBUF i16.
  - DVE fused WPACK per channel: W = max(|s|,|d|)*8192 + |s-d| + (|s|>=|d|)
      = mag*8192 + 2*ay + ss   (exact in f32: < 2^24)
    where mag=|gx|+|gy|, ay=|gy|, ss=(gx*gy>=0) of that channel.
  - STT maxes with +2048 prio bias -> exact first-wins channel argmax.
  - Unpack (mag / yb / ss / ay), fused SECTOR op -> q = 2*horiz + vert.
  - NMS: in-place copy_predicated chains pick the sector's neighbors
    from DMA-shifted mag rows; fused gates keep mag iff local max.
  - Pack strong/weak bitplanes via matmul; packed hysteresis; unpack.
"""
import sys
sys.path.insert(0, '/opt/trn_rl_repo')
from contextlib import ExitStack
import numpy as np
import ml_dtypes

import concourse.bass as bass
import concourse.tile as tile
from concourse import bacc, mybir

F32 = mybir.dt.float32
BF16 = mybir.dt.bfloat16
I16 = mybir.dt.int16
I32 = mybir.dt.int32

OP = mybir.AluOpType
AF = mybir.ActivationFunctionType

STRIDE = 120          # valid mask rows per tile
TILE_R = 128          # input rows per tile
PACK = 24             # rows per packed int32 word
WPT = STRIDE // PACK  # words per tile = 5
OUT_TILE = 120        # output rows per unpack tile

BF = ml_dtypes.bfloat16

SEC_C1 = float(np.float32(1.0 - 1.0 / np.sqrt(2.0)))  # tan22/(1+tan22)
SEC_C2 = float(np.float32(1.0 / np.sqrt(2.0)))        # tan67/(1+tan67)

# ---------------- custom DVE ops (registered on import) ----------------

from concourse.dve_spec import (Spec, Src0, Src1, C0, C1, C2, Zero, One,
                                maxx, select, eq, lower, Bin)
from concourse.dve_uop import AluOp
from concourse.dve_ops import DveOp, OPS, DveOpSpec
import concourse.dve_ops as _dvo


def _ABS(x):
    return Bin(AluOp.ABSOLUTE_VALUE, x, Zero)


def _ADIFF(a, b):
    return Bin(AluOp.ABSOLUTE_DIFF, a, b)


def _mk(name, spec):
    op = DveOp(name, spec, subdim=False, uops_sha={})
    for ver in ("v3", "v4"):
        try:
            s = DveOpSpec(name=name, opcode=0, uops=lower(spec, ver=ver),
                          rd1_en=True)
            op.uops_sha[ver] = s.sha(ver)
        except Exception:
            pass
    return op


def _register():
    _t = (Src0 + One) * C0
    _r = (_t + C1) - C1
    u8floor = _mk("U8FLOOR_ANT", Spec(
        body=_r - (_r > _t),
        reference=lambda in0, in1, s0, s1, imm2: np.floor(
            (in0.astype(np.float32) + np.float32(1.0)) * np.float32(s0)),
    ))
    _as = _ABS(Src0)
    _ad = _ABS(Src1)
    _m = maxx(_as, _ad)
    wpack = _mk("WPACK_ANT", Spec(
        body=_m * C0 + (_ADIFF(Src0, Src1) + eq(_m, _as)),
        reference=lambda in0, in1, s0, s1, imm2: (
            np.maximum(np.abs(in0), np.abs(in1)) * np.float32(s0)
            + np.abs(in0 - in1)
            + (np.abs(in0) >= np.abs(in1)).astype(np.float32)),
    ))
    _h = Src0 < Src1 * C0
    sector = _mk("SECTOR_ANT", Spec(
        body=(_h + _h) + (Src0 > Src1 * C1),
        reference=lambda in0, in1, s0, s1, imm2: (
            (in0 < in1 * np.float32(s0)).astype(np.float32) * 2.0
            + (in0 > in1 * np.float32(s1)).astype(np.float32)),
    ))
    gate_gt = _mk("GATE_GT_ANT", Spec(
        body=select(Src0 > Src1, Src0, Zero),
        reference=lambda in0, in1, s0, s1, imm2: np.where(
            in0 > in1, in0, 0.0),
    ))
    gate_ge = _mk("GATE_GE_ANT", Spec(
        body=select(Src0 >= Src1, Src0, Zero),
        reference=lambda in0, in1, s0, s1, imm2: np.where(
            in0 >= in1, in0, 0.0),
    ))
    ops = [u8floor, wpack, sector, gate_gt, gate_ge]
    for op in ops:
        if not any(o.name == op.name for o in OPS):
            OPS.append(op)
            _dvo.CUSTOM_DVE_SPECS[op.name] = op.spec
            _dvo._SUB_OPCODE_FOR_NAME[op.name] = (
                _dvo._CUSTOM_DVE_ROW_BASE + len(OPS) - 1)
    assert max(_dvo._SUB_OPCODE_FOR_NAME.values()) < 0x20
    return {o.name: o for o in OPS}


_OPMAP = _register()
U8FLOOR = _OPMAP["U8FLOOR_ANT"]
WPACK = _OPMAP["WPACK_ANT"]
SECTOR = _OPMAP["SECTOR_ANT"]
GATE_GT = _OPMAP["GATE_GT_ANT"]
GATE_GE = _OPMAP["GATE_GE_ANT"]


# ---------------- consts ----------------

def ext_rows(T):
    return STRIDE * (T - 1) + TILE_R  # xs shard rows


def make_consts(T=18):
    WORDS = WPT * T
    WPAD = WORDS + 6
    # s/d conv stationaries, lhsT layout: out[m] = sum_k lhsT[k,m] u[k].
    # Ks = Kx+Ky, Kd = Kx-Ky; column j holds the vertical taps for
    # horizontal offset j (rhs view shifted by j): S_j[k,m] = K[k-m+1, j].
    Ks = np.array([[-2., -2., 0.], [-2., 0., 2.], [0., 2., 2.]])
    Kd = np.array([[0., 2., 2.], [-2., 0., 2.], [-2., -2., 0.]])
    sobn = np.zeros((128, 6 * 128), np.float32)
    for b, K in ((0, Ks), (1, Kd)):
        for j in range(3):
            col = (b * 3 + j) * 128
            for m in range(128):
                for di in (-1, 0, 1):
                    k = m + di
                    if 0 <= k < 128:
                        sobn[k, col + m] = K[di + 1, j]
    # pack stationaries (same as baseline)
    p24 = np.zeros((128, T * WPAD), np.float32)
    for t in range(T):
        for p in range(2, 122):
            s = p - 2
            p24[p, t * WPAD + WPT * t + s // PACK] = float(1 << (s % PACK))
    # unpack one-hot: [6, 3*128]
    mrep = np.zeros((6, 3 * 128), np.float32)
    pat = np.zeros((128, 512), np.int32)
    for p in range(128):
        sp = 4 + p
        w, r = sp // PACK, sp % PACK
        j, k = r // 8, r % 8
        mrep[w, j * 128 + p] = 1.0
        pat[p, :] = 1 << k
    return {"sobn": sobn.astype(BF), "p24": p24.astype(BF),
            "mrep": mrep.astype(BF), "pat": pat}


# ---------------- kernel builder ----------------

def build_canny(T=18, rows_out=2048, hyst_iters=3):
    EXT = ext_rows(T)
    WORDS = WPT * T
    WPAD = WORDS + 6
    assert WPAD <= 128
    n_out_tiles = (rows_out + OUT_TILE - 1) // OUT_TILE

    nc = bacc.Bacc("TRN2", target_bir_lowering=False, debug=False,
                   num_devices=8)
    xs = nc.dram_tensor("xs", [3, EXT, 514], F32, kind="ExternalInput").ap()
    sobn = nc.dram_tensor("sobn", [128, 6 * 128], BF16,
                          kind="ExternalInput").ap()
    p24 = nc.dram_tensor("p24", [128, T * WPAD], BF16,
                         kind="ExternalInput").ap()
    mrep = nc.dram_tensor("mrep", [6, 3 * 128], BF16,
                          kind="ExternalInput").ap()
    pat = nc.dram_tensor("pat", [128, 512], I32, kind="ExternalInput").ap()
    rv = nc.dram_tensor("rv", [128, 2], F32, kind="ExternalInput").ap()
    out = nc.dram_tensor("out", [rows_out, 512], F32,
                         kind="ExternalOutput").ap()

    with tile.TileContext(nc) as tc:
        with ExitStack() as octx:
            cpool = octx.enter_context(tc.tile_pool(name="consts", bufs=1))
            sobn_b = cpool.tile([128, 6 * 128], BF16, tag="sobnb")
            nc.sync.dma_start(sobn_b[:], sobn[:, :])
            p24_b = cpool.tile([128, T * WPAD], BF16, tag="p24b")
            nc.sync.dma_start(p24_b[:], p24[:, :])
            mrep_b = cpool.tile([6, 3 * 128], BF16, tag="mrepb")
            nc.sync.dma_start(mrep_b[:], mrep[:, :])
            pat_s = cpool.tile([128, 512], I32, tag="pats")
            nc.sync.dma_start(pat_s[:], pat[:, :])
            rv_s = cpool.tile([128, 2], F32, tag="rvs")
            nc.sync.dma_start(rv_s[:], rv[:, :])

            pk = octx.enter_context(
                tc.tile_pool(name="packps", bufs=1, space="PSUM"))
            mmS = pk.tile([WPAD, 512], F32, tag="mmS")
            mmW = pk.tile([WPAD, 512], F32, tag="mmW")

            # ============ phase A (2-tile pairs) ============
            with ExitStack() as actx:
                xin_p = actx.enter_context(tc.tile_pool(name="xin", bufs=3))
                u_p = actx.enter_context(tc.tile_pool(name="up", bufs=3))
                sd_ps = actx.enter_context(
                    tc.tile_pool(name="sdps", bufs=6, space="PSUM"))
                sd_p = actx.enter_context(tc.tile_pool(name="sdp", bufs=2))
                w_p = actx.enter_context(tc.tile_pool(name="wp", bufs=2))
                sm_p = actx.enter_context(tc.tile_pool(name="smp", bufs=5))
                mg_p = actx.enter_context(tc.tile_pool(name="mgp", bufs=2))
                sh_p = actx.enter_context(tc.tile_pool(name="shp", bufs=2))
                nb_p = actx.enter_context(tc.tile_pool(name="nbp", bufs=4))
                st_p = actx.enter_context(tc.tile_pool(name="stp", bufs=2))

                for w_ in range(14):
                    nc.tensor.matmul(mmS[:, 0:512], sobn_b[:, 0:96],
                                     p24_b[:, 0:512], start=True, stop=True)
                for tp in range(T // 2):
                    t0 = 2 * tp
                    xin = xin_p.tile([128, 2 * 3 * 514], F32, tag="xin",
                                     name=f"xin{tp}")
                    for i in range(2):
                        a = STRIDE * (t0 + i)
                        half = xin[:, i * 1542:(i + 1) * 1542]
                        nc.sync.dma_start(
                            half.rearrange("p (c w) -> p c w", c=3),
                            xs[:, a:a + 128, :].rearrange("c p w -> p c w"))
                    # u = floor(fl(fl(x+1)*127.5)) in one fused op
                    u = u_p.tile([128, 2 * 3 * 514], BF16, tag="u",
                                 name=f"u{tp}")
                    nc.vector._custom_dve(U8FLOOR, out=u[:], in0=xin[:],
                                          s0=127.5, s1=float(2 ** 23))
                    # PE: s,d per (tile, channel) via 3 shifted matmuls each.
                    # sde layout: [s_c0t0 s_c0t1 s_c1t0 .. | d_c0t0 ..] so
                    # WPACK channel blocks are 1024 wide.
                    sde = sd_p.tile([128, 4 * 3 * 512], I16, tag="sde",
                                    name=f"sde{tp}")
                    for b in range(2):
                        for c in range(3):
                            for i in range(2):
                                ps = sd_ps.tile([128, 512], F32, tag="ps",
                                                name=f"ps{b}{c}{i}_{tp}")
                                for j in range(3):
                                    col = (b * 3 + j) * 128
                                    u0 = i * 1542 + c * 514 + j
                                    nc.tensor.matmul(
                                        ps[:], sobn_b[:, col:col + 128],
                                        u[:, u0:u0 + 512],
                                        start=(j == 0), stop=(j == 2))
                                nc.scalar.copy(
                                    sde[:, (b * 6 + c * 2 + i) * 512:
                                        (b * 6 + c * 2 + i + 1) * 512], ps[:])
                    # W_c = mag*8192 + 2*ay + ss  (f32, exact)
                    W = w_p.tile([128, 6 * 512], F32, tag="W", name=f"W{tp}")
                    nc.vector._custom_dve(
                        WPACK, out=W[:], in0=sde[:, 0:3072],
                        in1=sde[:, 3072:6144], s0=8192.0)
                    # channel argmax with first-wins prio bias
                    Wm = sm_p.tile([128, 1024], F32, tag="sm", name=f"Wm{tp}")
                    nc.vector.scalar_tensor_tensor(
                        out=Wm[:], in0=W[:, 0:1024], scalar=2048.0,
                        in1=W[:, 1024:2048], op0=OP.add, op1=OP.max)
                    Wx = sm_p.tile([128, 1024], F32, tag="sm", name=f"Wx{tp}")
                    nc.vector.scalar_tensor_tensor(
                        out=Wx[:], in0=Wm[:], scalar=2048.0,
                        in1=W[:, 2048:3072], op0=OP.add, op1=OP.max)
                    if tp == 0 or tp == T // 2 - 1:
                        blk = 0 if tp == 0 else 1
                        i = 0 if tp == 0 else 1
                        Wz = sm_p.tile([128, 1024], F32, tag="sm",
                                       name=f"Wz{tp}")
                        nc.vector.tensor_copy(
                            Wz[:, (1 - i) * 512:(2 - i) * 512],
                            Wx[:, (1 - i) * 512:(2 - i) * 512])
                        nc.vector.tensor_scalar(
                            out=Wz[:, i * 512:(i + 1) * 512],
                            in0=Wx[:, i * 512:(i + 1) * 512],
                            scalar1=rv_s[:, blk:blk + 1], scalar2=None,
                            op0=OP.mult)
                        Wx = Wz
                    wi = sm_p.tile([128, 1024], I32, tag="sm", name=f"wi{tp}")
                    nc.scalar.copy(wi[:], Wx[:])
                    # unpack: mag into halo'd row tensor, ss, ay
                    magp = mg_p.tile([128, 2 * 514], I32, tag="magp",
                                     name=f"magp{tp}")
                    for i in range(2):
                        nc.gpsimd.memset(magp[:, i * 514:i * 514 + 1], 0)
                        nc.gpsimd.memset(
                            magp[:, i * 514 + 513:i * 514 + 514], 0)
                    magv = magp[:].rearrange("p (i w) -> p i w", i=2)
                    nc.vector.tensor_scalar(
                        out=magv[:, :, 1:513], in0=wi[:], scalar1=13,
                        scalar2=None, op0=OP.logical_shift_right)
                    sst = sm_p.tile([128, 1024], I32, tag="sm",
                                    name=f"sst{tp}")
                    nc.vector.tensor_scalar(out=sst[:], in0=wi[:], scalar1=1,
                                            scalar2=None, op0=OP.bitwise_and)
                    ayt = sm_p.tile([128, 1024], I32, tag="sm",
                                    name=f"ayt{tp}")
                    nc.vector.tensor_scalar(out=ayt[:], in0=wi[:], scalar1=1,
                                            scalar2=1023,
                                            op0=OP.logical_shift_right,
                                            op1=OP.bitwise_and)
                    q = sm_p.tile([128, 1024], I16, tag="smq", name=f"q{tp}")
                    nc.vector._custom_dve(SECTOR, out=q[:], in0=ayt[:],
                                          in1=magv[:, :, 1:513],
                                          s0=SEC_C1, s1=SEC_C2)
                    hmt = sm_p.tile([128, 1024], I16, tag="smq",
                                    name=f"hmt{tp}")
                    nc.vector.tensor_scalar(out=hmt[:], in0=q[:], scalar1=2,
                                            scalar2=None, op0=OP.bitwise_and)
                    vmt = sm_p.tile([128, 1024], I16, tag="smq",
                                    name=f"vmt{tp}")
                    nc.vector.tensor_scalar(out=vmt[:], in0=q[:], scalar1=1,
                                            scalar2=None, op0=OP.bitwise_and)
                    # neighbor rows via SBUF partition-shift DMA (both tiles)
                    mu = sh_p.tile([128, 2 * 514], I32, tag="mu",
                                   name=f"mu{tp}")
                    nc.gpsimd.memset(mu[96:128, :], 0)
                    nc.sync.dma_start(mu[0:127, :], magp[1:128, :])
                    md = sh_p.tile([128, 2 * 514], I32, tag="md",
                                   name=f"md{tp}")
                    nc.gpsimd.memset(md[0:32, :], 0)
                    nc.sync.dma_start(md[1:128, :], magp[0:127, :])
                    muv = mu[:].rearrange("p (i w) -> p i w", i=2)
                    mdv = md[:].rearrange("p (i w) -> p i w", i=2)
                    # nb: default UR, then d1->UL, vert->U, horiz->L
                    nb = nb_p.tile([128, 1024], I32, tag="nb", name=f"nb{tp}")
                    nbv = nb[:].rearrange("p (i w) -> p i w", i=2)
                    nc.sync.dma_start(nbv, mdv[:, :, 2:514])
                    nc.vector.copy_predicated(nbv, sst[:], mdv[:, :, 0:512])
                    nc.vector.copy_predicated(nbv, vmt[:], mdv[:, :, 1:513])
                    nc.vector.copy_predicated(nbv, hmt[:], magv[:, :, 0:512])
                    # na: default DL, then d1->DR, vert->D, horiz->R
                    na = nb_p.tile([128, 1024], I32, tag="nb", name=f"na{tp}")
                    nav = na[:].rearrange("p (i w) -> p i w", i=2)
                    nc.sync.dma_start(nav, muv[:, :, 0:512])
                    nc.vector.copy_predicated(nav, sst[:], muv[:, :, 2:514])
                    nc.vector.copy_predicated(nav, vmt[:], muv[:, :, 1:513])
                    nc.vector.copy_predicated(nav, hmt[:], magv[:, :, 2:514])
                    # gates: km = mag if (mag>nb) & (mag>=na) else 0
                    km1 = st_p.tile([128, 1024], I32, tag="km",
                                    name=f"km1{tp}")
                    nc.vector._custom_dve(GATE_GT, out=km1[:],
                                          in0=magv[:, :, 1:513], in1=nb[:])
                    km = st_p.tile([128, 1024], I32, tag="km", name=f"km{tp}")
                    nc.vector._custom_dve(GATE_GE, out=km[:], in0=km1[:],
                                          in1=na[:])
                    st = st_p.tile([128, 1024], BF16, tag="st",
                                   name=f"st{tp}")
                    nc.vector.tensor_scalar(out=st[:], in0=km[:],
                                            scalar1=200.0, scalar2=None,
                                            op0=OP.is_gt)
                    wk = st_p.tile([128, 1024], BF16, tag="st",
                                   name=f"wk{tp}")
                    nc.vector.tensor_scalar(out=wk[:], in0=km[:],
                                            scalar1=100.0, scalar2=None,
                                            op0=OP.is_gt)
                    for i in range(2):
                        t = t0 + i
                        lhs = p24_b[:, t * WPAD:(t + 1) * WPAD]
                        nc.tensor.matmul(mmS[:], lhs,
                                         st[:, i * 512:(i + 1) * 512],
                                         start=(t == 0), stop=(t == T - 1))
                        nc.tensor.matmul(mmW[:], lhs,
                                         wk[:, i * 512:(i + 1) * 512],
                                         start=(t == 0), stop=(t == T - 1))

            # ============ phase B: packed hysteresis ============
            with ExitStack() as bctx:
                hw_ = bctx.enter_context(tc.tile_pool(name="hw", bufs=1))
                it_p = bctx.enter_context(tc.tile_pool(name="itp", bufs=2))
                sW = hw_.tile([WPAD, 512], I32, tag="sW")
                nc.vector.tensor_copy(sW[:], mmW[:])
                cur = hw_.tile([WPAD, 512], I32, tag="cur0")
                nc.vector.tensor_copy(cur[:], mmS[:])
                for it in range(hyst_iters):
                    uw = it_p.tile([WPAD, 512], I32, tag="uw", name=f"uw{it}")
                    nc.gpsimd.memset(uw[(WPAD - 1) // 32 * 32:WPAD], 0)
                    nc.sync.dma_start(uw[0:WPAD - 1], cur[1:WPAD])
                    dw = it_p.tile([WPAD, 512], I32, tag="dw", name=f"dw{it}")
                    nc.gpsimd.memset(dw[0:min(32, WPAD)], 0)
                    nc.sync.dma_start(dw[1:WPAD], cur[0:WPAD - 1])
                    sl = it_p.tile([WPAD, 512], I32, tag="sl", name=f"sl{it}")
                    nc.vector.tensor_scalar(out=sl[:], in0=cur[:], scalar1=1,
                                            scalar2=None,
                                            op0=OP.logical_shift_left)
                    sr = it_p.tile([WPAD, 512], I32, tag="sr", name=f"sr{it}")
                    nc.vector.tensor_scalar(out=sr[:], in0=cur[:], scalar1=1,
                                            scalar2=None,
                                            op0=OP.logical_shift_right)
                    cu = it_p.tile([WPAD, 512], I32, tag="cu", name=f"cu{it}")
                    nc.vector.tensor_scalar(out=cu[:], in0=dw[:],
                                            scalar1=PACK - 1, scalar2=None,
                                            op0=OP.logical_shift_right)
                    cd = it_p.tile([WPAD, 512], I32, tag="cd", name=f"cd{it}")
                    nc.vector.tensor_scalar(out=cd[:], in0=uw[:],
                                            scalar1=PACK - 1, scalar2=None,
                                            op0=OP.logical_shift_left)
                    o1 = it_p.tile([WPAD, 512], I32, tag="o1", name=f"o1_{it}")
                    nc.vector.tensor_tensor(out=o1[:], in0=sl[:], in1=sr[:],
                                            op=OP.bitwise_or)
                    o2 = it_p.tile([WPAD, 512], I32, tag="o2", name=f"o2_{it}")
                    nc.vector.tensor_tensor(out=o2[:], in0=cu[:], in1=cd[:],
                                            op=OP.bitwise_or)
                    o3 = it_p.tile([WPAD, 512], I32, tag="o3", name=f"o3_{it}")
                    nc.vector.tensor_tensor(out=o3[:], in0=o1[:], in1=o2[:],
                                            op=OP.bitwise_or)
                    vor = it_p.tile([WPAD, 512], I32, tag="vor",
                                    name=f"vor{it}")
                    nc.vector.tensor_tensor(out=vor[:], in0=o3[:],
                                            in1=cur[:], op=OP.bitwise_or)
                    q_ = it_p.tile([WPAD, 512], I32, tag="q", name=f"q{it}")
                    nc.vector.tensor_tensor(out=q_[:, 1:512],
                                            in0=vor[:, 0:511],
                                            in1=vor[:, 1:512],
                                            op=OP.bitwise_or)
                    nc.vector.tensor_copy(q_[:, 0:1], vor[:, 0:1])
                    r_ = it_p.tile([WPAD, 512], I32, tag="r", name=f"r{it}")
                    nc.vector.tensor_tensor(out=r_[:, 0:511], in0=q_[:, 0:511],
                                            in1=vor[:, 1:512],
                                            op=OP.bitwise_or)
                    nc.vector.tensor_copy(r_[:, 511:512], q_[:, 511:512])
                    ncur = hw_.tile([WPAD, 512], I32, tag=f"cur{it + 1}",
                                    name=f"ncur{it + 1}")
                    nc.vector.tensor_tensor(out=ncur[:], in0=r_[:],
                                            in1=sW[:], op=OP.bitwise_and)
                    cur = ncur
                # byte-split for the unpack matmul
                bi = []
                for j, (s1v, s2v, o0, o1v) in enumerate([
                        (255, None, OP.bitwise_and, None),
                        (8, 255, OP.logical_shift_right, OP.bitwise_and),
                        (16, 255, OP.logical_shift_right, OP.bitwise_and),
                ]):
                    x_ = hw_.tile([WPAD, 512], I32, tag=f"bi{j}",
                                  name=f"bi{j}")
                    if o1v is None:
                        nc.vector.tensor_scalar(out=x_[:], in0=cur[:],
                                                scalar1=s1v, scalar2=None,
                                                op0=o0)
                    else:
                        nc.vector.tensor_scalar(out=x_[:], in0=cur[:],
                                                scalar1=s1v, scalar2=s2v,
                                                op0=o0, op1=o1v)
                    bi.append(x_)
                b012 = hw_.tile([WPAD, 3 * 512], BF16, tag="b012")
                for j in range(3):
                    nc.vector.tensor_copy(b012[:, j * 512:(j + 1) * 512],
                                          bi[j][:])
                unp = bctx.enter_context(
                    tc.tile_pool(name="unp", bufs=2, space="PSUM"))
                uo_p = bctx.enter_context(tc.tile_pool(name="uo", bufs=3))
                no2 = (n_out_tiles + 1) // 2
                for op_ in range(no2):
                    o0 = 2 * op_
                    k = min(2, n_out_tiles - o0)
                    bs = uo_p.tile([6, 2 * 3 * 512], BF16, tag="bs",
                                   name=f"bs_{op_}")
                    psq = unp.tile([128, 2 * 512], F32, tag="ps",
                                   name=f"ps{op_}")
                    for i in range(k):
                        w0 = WPT * (o0 + i)
                        nc.sync.dma_start(
                            bs[:, i * 1536:(i + 1) * 1536],
                            b012[w0:w0 + 6, :])
                        for j in range(3):
                            nc.tensor.matmul(
                                psq[:, i * 512:(i + 1) * 512],
                                mrep_b[:, j * 128:(j + 1) * 128],
                                bs[:, i * 1536 + j * 512:
                                   i * 1536 + (j + 1) * 512],
                                start=(j == 0), stop=(j == 2))
                    w_ = k * 512
                    pse = uo_p.tile([128, 2 * 512], I32, tag="pse",
                                    name=f"pse{op_}")
                    nc.scalar.copy(pse[:, 0:w_], psq[:, 0:w_])
                    bits = uo_p.tile([128, 2 * 512], I32, tag="bits",
                                     name=f"bits{op_}")
                    bv = bits[:].rearrange("p (i w) -> p i w", i=2)
                    pv = pse[:].rearrange("p (i w) -> p i w", i=2)
                    nc.vector.tensor_tensor(
                        out=bv[:, 0:k], in0=pv[:, 0:k],
                        in1=pat_s[:].unsqueeze(1).broadcast_to([128, k, 512])
                        if k > 1 else pat_s[:].unsqueeze(1),
                        op=OP.bitwise_and)
                    ot = uo_p.tile([128, 2 * 512], F32, tag="ot",
                                   name=f"ot{op_}")
                    nc.vector.tensor_scalar(out=ot[:, 0:w_],
                                            in0=bits[:, 0:w_],
                                            scalar1=0, scalar2=255.0,
                                            op0=OP.not_equal, op1=OP.mult)
                    for i in range(k):
                        o = o0 + i
                        nrows = min(OUT_TILE, rows_out - o * OUT_TILE)
                        nc.sync.dma_start(
                            out[o * OUT_TILE:o * OUT_TILE + nrows, :],
                            ot[0:nrows, i * 512:(i + 1) * 512])

    nc.compile()
    return nc


# ---------------- host-side helpers ----------------

def shard_inputs(x, T=18, rows_out=2048, n_cores=8):
    B, C, H, W = x.shape
    NR = B * H
    WORDS = WPT * T
    WPAD = WORDS + 6
    tall = np.ascontiguousarray(x.transpose(1, 0, 2, 3).reshape(C, NR, W))
    tallp = np.pad(tall, ((0, 0), (0, 0), (1, 1)), mode='edge')
    EXT = ext_rows(T)
    consts = make_consts(T)
    maps = []
    for k in range(n_cores):
        r0 = k * rows_out - 6
        idx = np.clip(np.arange(r0, r0 + EXT), 0, NR - 1)
        shard = np.ascontiguousarray(tallp[:, idx, :])
        # per-core row validity for boundary tiles (tall row in [0, NR))
        rv = np.ones((128, 2), np.float32)
        for bi, t in ((0, 0), (1, T - 1)):
            rows = r0 + STRIDE * t + np.arange(128)
            bad = (rows < 0) | (rows >= NR)
            rv[bad, bi] = 0.0
        # per-core pack stationary: zero strip rows outside the image
        p24 = np.array(consts["p24"], dtype=np.float32)
        for t in range(T):
            for p in range(2, 122):
                g = k * rows_out - 4 + STRIDE * t + (p - 2)
                if g < 0 or g >= NR:
                    p24[p, t * WPAD + WPT * t + (p - 2) // PACK] = 0.0
        m = {"xs": shard, "rv": rv, "p24": p24.astype(BF)}
        m.update({kk: v for kk, v in consts.items() if kk != "p24"})
        maps.append(m)
    return maps


def assemble_output(results, B=32, H=512, W=512):
    outs = [r["out"] for r in results]
    tallout = np.concatenate(outs, axis=0)
    img = tallout.reshape(B, H, W)
    return np.broadcast_to(img[:, None], (B, 3, H, W))


# ---------------- harness entry point ----------------

_NC_CACHE = {}


def _get_nc():
    if "nc" not in _NC_CACHE:
        _NC_CACHE["nc"] = build_canny(T=18, rows_out=2048, hyst_iters=1)
    return _NC_CACHE["nc"]


def kernel(x):
    """Full-input entry point: x (32,3,512,512) f32 -> (32,3,512,512) f32."""
    from concourse.bass_utils import run_bass_kernel_spmd
    x = np.asarray(x, dtype=np.float32)
    nc = _get_nc()
    in_maps = shard_inputs(x, T=18, rows_out=2048, n_cores=8)
    res = run_bass_kernel_spmd(nc, in_maps, list(range(8)))
    out = assemble_output(res.results)
    return np.ascontiguousarray(out).astype(np.float32)
